# revision 20
# baseline (speedup 1.0000x reference)
"""Trainium2 Bass kernel for nn_Conv_8443905704574.

Reference semantics: 7x7 cross-correlation (stride 1, zero pad 3) applied to
the LAST input channel only; the single-channel result is broadcast to all 3
output channels.

Device algorithm: banded-Toeplitz matmul conv in bf16. For each 128-row input
block, the 7 kernel columns become 7 stationary [128,128] band matrices
(entries T[k,m] = K[k-m+off, dj]); each is matmul'd against a W-shifted slice
of the block, accumulating the 7 taps in fp32 PSUM. One block yields 122
valid output rows.

Perf notes (from NTFF/perfetto analysis of the fp32r version):
- HWDGE descriptor generation (~25ns/desc) on a single sequencer ring was the
  bottleneck, so input loads alternate between the two HWDGE rings (sync/SP
  and scalar/ACT) and output stores go through gpsimd (SWDGE, Q7-generated
  descriptors) as full-width 4KB-per-row transfers.
- fp32r matmuls pay a ~250ns non-overlapped LDWEIGHTS each; bf16 pipelines to
  ~131ns/MM. Inputs are converted to bf16 on the host (error ~0.3% << 2e-2
  tolerance); accumulation stays fp32 in PSUM.
- W zero-padding is done once on-device (memset) instead of a host-side
  padded copy; DMAs write only the interior columns so pads stay zero.

Sharding: pure data parallel — 2 images per core across 8 cores; host slices
the last channel, device computes [2,1024,1024], host broadcasts channels.
"""

import numpy as np

import concourse.bacc as bacc
import concourse.mybir as mybir
import concourse.tile as tile
from concourse.bass_utils import run_bass_kernel_spmd

B, C, H, W = 16, 3, 1024, 1024
KS = 7
PAD = KS // 2
NCORES = 8
PER = B // NCORES          # images per core
SEAM = 2 * PAD             # zero rows between stacked images
XH = PER * H + SEAM        # input strip height (images + seam)
TILE_OUT = 128 - (KS - 1)  # 122 valid output rows per H-tile
NT = (XH + TILE_OUT - 1) // TILE_OUT  # 17 blocks over the strip
WCH = 512                  # W chunk = one fp32 PSUM bank
NWCH = W // WCH            # 2
XW = W + 2 * PAD           # padded row width in SBUF
NXB = 8                    # input block buffers

f32 = mybir.dt.float32
bf16 = mybir.dt.bfloat16
BF16_NP = mybir.dt.np(bf16)

_CACHE = {}
LAST_RESULTS = None


def _build_bass():
    nc = bacc.Bacc("TRN2", target_bir_lowering=False, debug=False)
    x = nc.dram_tensor("x", [XH, W], bf16, kind="ExternalInput")
    tmat = nc.dram_tensor("tmat", [128, 2 * KS * 128], bf16, kind="ExternalInput")
    out = nc.dram_tensor("out", [PER, H, W], bf16, kind="ExternalOutput")

    with tile.TileContext(nc) as tc:
        with (
            tc.tile_pool(name="tmp", bufs=1) as tmpool,
            tc.tile_pool(name="xp", bufs=1) as xpool,
            tc.tile_pool(name="op", bufs=6) as opool,
            tc.tile_pool(name="pp", bufs=8, space="PSUM") as ppool,
            tc.tile_pool(name="wz", bufs=1) as wzpool,
        ):
            # PE warm-up during the DMA lead-in: zero matmuls keep the PE HAM
            # clock gate busy so real matmuls start at full clock (~3.4us of
            # issue time is what the HAM window wants).
            wz = wzpool.tile([128, 128 + WCH], bf16, name="wz")
            nc.vector.memset(wz[:], 0.0)
            # Enough warm-up to cover the first input block's DMA latency and
            # most of the PE HAM ramp (~3.4us of issue time).
            for i in range(8):
                pzt = ppool.tile([128, WCH], f32, name="pt", tag="pt")
                nc.tensor.matmul(
                    pzt[:],
                    wz[:, 0:128],
                    wz[:, 128 : 128 + WCH],
                    start=True, stop=True,
                )

            x_tiles = []
            for i in range(NXB):
                xt = xpool.tile([128, XW], bf16, name=f"xt{i}", tag=f"xt{i}")
                # One-time zero fill of just the pad columns (block loads only
                # ever write the interior, so the pads stay zero across reuse).
                nc.vector.memset(xt[:, 0:PAD], 0.0)
                nc.vector.memset(xt[:, PAD + W : XW], 0.0)
                x_tiles.append(xt)

            # first input block + band matrices: the critical path
            t_sb = tmpool.tile([128, 2 * KS * 128], bf16, name="t_sb")

            def tile_geo(t):
                # Geometry in strip coordinates (both images + zero seam).
                r0 = t * TILE_OUT
                nv = min(TILE_OUT, XH - r0)
                # First block starts at the strip edge (band offset PAD);
                # interior blocks start PAD rows above their outputs.
                if t == 0:
                    in0, variant = 0, 0
                else:
                    in0, variant = r0 - PAD, 1
                nk = min(128, XH - in0)
                return r0, nv, in0, nk, variant

            def out_segments(r0, nv):
                """Map strip output rows [r0, r0+nv) to per-image stores:
                (psum_row_offset, img, img_row0, n_rows). Seam rows are
                computed but never stored."""
                segs = []
                for img in range(PER):
                    lo = img * (H + SEAM)
                    s = max(r0, lo)
                    e = min(r0 + nv, lo + H)
                    if e > s:
                        segs.append((s - r0, img, s - lo, e - s))
                return segs

            def issue_load(t):
                _, _, in0, nk, _ = tile_geo(t)
                xt = x_tiles[t % NXB]
                # Alternate the two HWDGE rings so descriptor generation for
                # loads runs on both sequencers in parallel.
                eng = nc.sync if t % 2 == 0 else nc.scalar
                eng.dma_start(xt[0:nk, PAD : PAD + W], x[in0 : in0 + nk, :])

            # x0 on the sync ring, band matrices on the scalar ring: both
            # gate the first real matmul and load in parallel.
            issue_load(0)
            nc.scalar.dma_start(t_sb[:], tmat[:])

            for t in range(NT):
                r0, nv, in0, nk, variant = tile_geo(t)
                xt = x_tiles[t % NXB]
                if t + 1 < NT:
                    issue_load(t + 1)
                # Output staged in bf16: the DVE copy casts fp32 PSUM down,
                # halving store bytes (host upcasts on return).
                ot = opool.tile([128, W], bf16, name="ot", tag="ot")
                for c in range(NWCH):
                    pt = ppool.tile([128, WCH], f32, name="pt", tag="pt")
                    for dj in range(KS):
                        col = (variant * KS + dj) * 128
                        nc.tensor.matmul(
                            pt[:],
                            t_sb[0:nk, col : col + 128],
                            xt[0:nk, c * WCH + dj : c * WCH + dj + WCH],
                            start=(dj == 0),
                            stop=(dj == KS - 1),
                        )
                    nc.vector.tensor_copy(
                        ot[0:nv, c * WCH : (c + 1) * WCH], pt[0:nv, :]
                    )
                # Full-width stores via SWDGE: one descriptor per output row,
                # written by the Q7s, spreading stores across SDMA engines
                # and keeping the HWDGE rings free for input loads. The final
                # block's stores go out on the (by then idle) HWDGE rings so
                # they flush in parallel with the SWDGE backlog.
                for si, (po, img, orow, n) in enumerate(out_segments(r0, nv)):
                    if t == NT - 1:
                        seng = nc.sync if si % 2 == 0 else nc.scalar
                        seng.dma_start(
                            out[img, orow : orow + n, :], ot[po : po + n, :]
                        )
                    else:
                        nc.gpsimd.dma_start(
                            out[img, orow : orow + n, :], ot[po : po + n, :]
                        )
    nc.compile()
    return nc


def _toeplitz(kmat: np.ndarray) -> np.ndarray:
    """[128, 2*KS*128] stationary band matrices: variant 0 = first block
    (band offset PAD), variant 1 = interior blocks (band offset 0)."""
    k_idx = np.arange(128)[:, None]
    m_idx = np.arange(128)[None, :]
    t_all = np.zeros((128, 2, KS, 128), dtype=np.float32)
    for variant, off in ((0, PAD), (1, 0)):
        di = k_idx - m_idx + off
        mask = (di >= 0) & (di < KS)
        dic = np.clip(di, 0, KS - 1)
        for dj in range(KS):
            t_all[:, variant, dj, :] = np.where(mask, kmat[dic, dj], 0.0)
    return t_all.reshape(128, 2 * KS * 128).astype(BF16_NP)


def _shard_inputs(image: np.ndarray, kmat: np.ndarray):
    tmat = _toeplitz(kmat)
    imgs = image[:, C - 1, :, :].reshape(NCORES, PER, H, W)
    xs = np.empty((NCORES, XH, W), dtype=BF16_NP)
    for img in range(PER):
        lo = img * (H + SEAM)
        xs[:, lo : lo + H] = imgs[:, img]  # casts f32 -> bf16 on assignment
        if img + 1 < PER:
            xs[:, lo + H : lo + H + SEAM] = 0.0
    return [{"x": xs[i], "tmat": tmat} for i in range(NCORES)]


def _cached_runner(nc):
    """Build (once) a jitted SPMD executor for `nc`, mirroring
    bass2jax.run_bass_via_pjrt but reusable across kernel() calls — the
    per-call jax.jit re-trace there costs ~2s of host wall per invocation.
    """
    import jax
    from jax.sharding import Mesh, PartitionSpec
    from jax.experimental.shard_map import shard_map
    import concourse.mybir as mybir_
    from concourse import bass2jax

    bass2jax.install_neuronx_cc_hook()

    partition_name = (
        nc.partition_id_tensor.name if nc.partition_id_tensor else None
    )
    in_names, out_names, out_avals, zero_shapes = [], [], [], []
    for alloc in nc.m.functions[0].allocations:
        if not isinstance(alloc, mybir_.MemoryLocationSet):
            continue
        name = alloc.memorylocations[0].name
        if alloc.kind == "ExternalInput":
            if name != partition_name:
                in_names.append(name)
        elif alloc.kind == "ExternalOutput":
            shape = tuple(alloc.tensor_shape)
            dtype = mybir_.dt.np(alloc.dtype)
            out_names.append(name)
            out_avals.append(jax.core.ShapedArray(shape, dtype))
            zero_shapes.append((shape, dtype))
    n_params = len(in_names)
    all_names = list(in_names) + list(out_names)
    if partition_name is not None:
        all_names.append(partition_name)
    donate = tuple(range(n_params, n_params + len(out_names)))

    def _body(*args):
        operands = list(args)
        if partition_name is not None:
            operands.append(bass2jax.partition_id_tensor())
        outs = bass2jax._bass_exec_p.bind(
            *operands,
            out_avals=tuple(out_avals),
            in_names=tuple(all_names),
            out_names=tuple(out_names),
            lowering_input_output_aliases=(),
            sim_require_finite=True,
            sim_require_nnan=True,
            nc=nc,
        )
        return tuple(outs)

    devices = jax.devices()[:NCORES]
    mesh = Mesh(np.asarray(devices), ("core",))
    in_specs = (PartitionSpec("core"),) * (n_params + len(out_names))
    out_specs = (PartitionSpec("core"),) * len(out_names)
    sharded = jax.jit(
        shard_map(
            _body, mesh=mesh, in_specs=in_specs, out_specs=out_specs,
            check_rep=False,
        ),
        donate_argnums=donate,
        keep_unused=True,
    )

    def run(in_maps):
        concat_in = [
            np.concatenate([np.asarray(m[name]) for m in in_maps], axis=0)
            for name in in_names
        ]
        concat_zeros = [
            np.zeros((NCORES * s[0], *s[1:]), d) for s, d in zero_shapes
        ]
        out_arrs = sharded(*concat_in, *concat_zeros)
        return {
            name: np.asarray(out_arrs[i]) for i, name in enumerate(out_names)
        }

    return run


def kernel(**inputs):
    global LAST_RESULTS
    image = np.asarray(inputs["image"], dtype=np.float32)
    kmat = np.asarray(inputs["kernel"], dtype=np.float32)
    assert image.shape == (B, C, H, W), image.shape

    if "nc" not in _CACHE:
        _CACHE["nc"] = _build_bass()
        _CACHE["run"] = _cached_runner(_CACHE["nc"])

    in_maps = _shard_inputs(image, kmat)
    outs = _CACHE["run"](in_maps)
    LAST_RESULTS = outs

    y = outs["out"].reshape(B, 1, H, W).astype(np.float32)
    return np.broadcast_to(y, (B, C, H, W))
